# revision 1
# baseline (speedup 1.0000x reference)
"""Causal multi-head attention on 8 Trainium2 NeuronCores.

v2: (batch x head-group) sharding — core c handles batch b=c//2 and heads
[8g:8g+8] (g=c%2), i.e. 4 row-packed head PAIRS per core. All matmul
operands are bf16 (fp32r streams at half the PE rate; bf16 runs 1 col/cycle
at 2.4GHz once the p-state ramps). PSUM accumulates in f32.

Per-core layout (partition dim first):
  t_x    [128, 8, 2048]  x_b^T tiled: [p, kd, seq], d = kd*128+p
  t_w    [128, 8, 1536]  wqkv tiles: [p, kd, q512|k512|v512]
  qT/kT  [128, 4, 2048]  pair m: partitions = m's 128 head dims
  vplus  [128, 16, 520]  per key tile: 8 x [v_head(64) | ones(1)]
  scores [128, 1024]     psum, both heads of a pair (row-packed matmuls)
  ctx    [65, 512]       psum per head: rows 0:64 ctx^T, row 64 denom
  ctxT   [128, 4, 2048]  normalized context, bf16
  out    [2048, 1024]    f32 partial (host sums 2 cores + bo per batch)

V is computed directly in transposed orientation (x-tile stationary,
wv moving) so no on-chip transposes are needed; the v bias is folded in
as a K=1 ones-row matmul that initializes the psum accumulator. q/k biases
fold into the psum->SBUF copies (per-partition tensor_scalar_add).

Emission order software-pipelines the attention inner loop (scores kt+1
issued before ctx kt) and weaves projection units for q-chunk qc+1 into
the attention stream of q-chunk qc to keep the tensor engine dense.
"""

import numpy as np
from contextlib import ExitStack

import concourse.bass as bass
import concourse.mybir as mybir
import concourse.tile as tile
from concourse import bacc
from concourse import bass_utils

F32 = mybir.dt.float32
BF16 = mybir.dt.bfloat16
AF = mybir.ActivationFunctionType

B, S, D = 4, 2048, 1024
H, DH = 16, 64
NCORES = 8
HG = 512            # head dims per core (8 heads)
NP = 4              # head pairs per core
QC = 512            # q-chunk width
NQC = S // QC       # 4
NKD = D // 128      # 8 contraction tiles
NKT = S // 128      # 16 key tiles

_CACHE = {}


def _build():
    nc = bacc.Bacc("TRN2", target_bir_lowering=False, debug=False)
    xt = nc.dram_tensor("xt", [128, NKD, S], BF16, kind="ExternalInput").ap()
    wqkv = nc.dram_tensor("wqkv", [128, NKD, 3 * HG], BF16, kind="ExternalInput").ap()
    bqk = nc.dram_tensor("bqk", [128, NP, 2], F32, kind="ExternalInput").ap()
    bv = nc.dram_tensor("bv", [1, HG], F32, kind="ExternalInput").ap()
    wo = nc.dram_tensor("wo", [128, NP, D], BF16, kind="ExternalInput").ap()
    cmask = nc.dram_tensor("cmask", [128, 4, 2 * QC], BF16, kind="ExternalInput").ap()
    out = nc.dram_tensor("out", [S, D], F32, kind="ExternalOutput").ap()

    with tile.TileContext(nc) as tc:
        with ExitStack() as ctx:
            consts = ctx.enter_context(tc.tile_pool(name="consts", bufs=1))
            expp = ctx.enter_context(tc.tile_pool(name="expp", bufs=6))
            small = ctx.enter_context(tc.tile_pool(name="small", bufs=2))
            ostage = ctx.enter_context(tc.tile_pool(name="ostage", bufs=2))
            psc = ctx.enter_context(tc.tile_pool(name="psc", bufs=2, space="PSUM"))
            pctx = ctx.enter_context(tc.tile_pool(name="pctx", bufs=3, space="PSUM"))
            pp = ctx.enter_context(tc.tile_pool(name="pp", bufs=1, space="PSUM"))

            # ---- persistent SBUF tensors ----
            t_x = consts.tile([128, NKD, S], BF16, tag="x")
            t_w = consts.tile([128, NKD, 3 * HG], BF16, tag="w")
            t_bqk = consts.tile([128, NP, 2], F32, tag="bqk")
            t_bv = consts.tile([1, HG], F32, tag="bv")
            t_onesc = consts.tile([128, 1], F32, tag="onesc")
            t_bvb = consts.tile([128, HG], F32, tag="bvb")
            t_wo = consts.tile([128, NP, D], BF16, tag="wo")
            t_mask = consts.tile([128, 4, 2 * QC], BF16, tag="mask")
            qT = consts.tile([128, NP, S], BF16, tag="qT")
            kT = consts.tile([128, NP, S], BF16, tag="kT")
            vplus = consts.tile([128, NKT, 8 * 65], BF16, tag="vplus")
            ctxT = consts.tile([128, NP, S], BF16, tag="ctxT")

            # input DMAs: weights first (needed in full by proj(0)), then
            # x chunk 0, then the remaining x chunks.
            nc.sync.dma_start(t_w[:, 0:4, :], wqkv[:, 0:4, :])
            nc.sync.dma_start(t_x[:, 0:4, 0:QC], xt[:, 0:4, 0:QC])
            nc.sync.dma_start(t_w[:, 4:8, :], wqkv[:, 4:8, :])
            nc.sync.dma_start(t_x[:, 4:8, 0:QC], xt[:, 4:8, 0:QC])
            nc.sync.dma_start(t_bqk, bqk)
            nc.sync.dma_start(t_bv, bv)
            nc.sync.dma_start(t_mask, cmask)
            nc.sync.dma_start(t_wo, wo)
            for qc in range(1, NQC):
                nc.sync.dma_start(
                    t_x[:, :, qc * QC:(qc + 1) * QC], xt[:, :, qc * QC:(qc + 1) * QC]
                )
            nc.vector.memset(t_onesc, 1.0)
            # broadcast the v bias to all partitions once (keys dim)
            nc.gpsimd.partition_broadcast(t_bvb, t_bv)
            # ones column after each head's 64 v-dims in every key tile
            for hb in range(8):
                nc.vector.tensor_copy(
                    vplus[:, :, 65 * hb + 64:65 * hb + 65],
                    t_onesc[:, None, :].broadcast_to([128, NKT, 1]),
                )

            # ---------- unit emitters ----------
            def proj_qk_unit(m, p, qc, pool, tag):
                ps = pool.tile([128, QC], F32, tag=tag, name="ps_p")
                c0 = p * HG + m * 128
                for kd in range(NKD):
                    nc.tensor.matmul(
                        ps, t_w[:, kd, c0:c0 + 128],
                        t_x[:, kd, qc * QC:(qc + 1) * QC],
                        start=(kd == 0), stop=(kd == NKD - 1),
                    )
                dst = (qT if p == 0 else kT)[:, m, qc * QC:(qc + 1) * QC]
                nc.vector.tensor_scalar_add(dst, ps, t_bqk[:, m, p:p + 1])

            def proj_v_unit(t, qc, pool, tag):
                kt = 4 * qc + t
                ps = pool.tile([128, QC], F32, tag=tag, name="ps_v")
                for kd in range(NKD):
                    nc.tensor.matmul(
                        ps, t_x[:, kd, kt * 128:(kt + 1) * 128],
                        t_w[:, kd, 2 * HG:3 * HG],
                        start=(kd == 0), stop=(kd == NKD - 1),
                    )
                dst = vplus[:, kt, :].rearrange("p (h c) -> p h c", h=8)[:, :, 0:64]
                nc.vector.tensor_add(
                    dst, ps.rearrange("p (h c) -> p h c", h=8),
                    t_bvb.rearrange("p (h c) -> p h c", h=8),
                )

            def proj_units(qc, pool_alt=False):
                units = []
                for m in range(NP):
                    for p in range(2):
                        units.append((proj_qk_unit, m, p, qc))
                for t in range(4):
                    units.append((proj_v_unit, t, qc))
                out_units = []
                for i, u in enumerate(units):
                    fn, *args = u
                    if pool_alt and i % 2 == 1:
                        pool, tag = psc, "s"
                    else:
                        pool, tag = pp, "p"
                    out_units.append(lambda fn=fn, args=args, pool=pool, tag=tag:
                                     fn(*args, pool, tag))
                return out_units

            def attn_scores_unit(g, qc, kt, st):
                ps_s = psc.tile([128, 2 * QC], F32, tag="s", name="ps_s")
                nc.tensor.matmul(
                    ps_s[:, 0:QC],
                    kT[0:64, g, kt * 128:(kt + 1) * 128],
                    qT[0:64, g, qc * QC:(qc + 1) * QC],
                    start=True, stop=True,
                )
                nc.tensor.matmul(
                    ps_s[:, QC:2 * QC],
                    kT[64:128, g, kt * 128:(kt + 1) * 128],
                    qT[64:128, g, qc * QC:(qc + 1) * QC],
                    start=True, stop=True, tile_position=(64, 0),
                )
                e = expp.tile([128, 2 * QC], BF16, tag="exp", name="t_e")
                nc.scalar.activation(e, ps_s, AF.Exp, scale=0.125)
                o = kt - 4 * qc
                if o >= 0:
                    nc.vector.tensor_mul(e, e, t_mask[:, o, :])
                st[kt] = e

            def attn_ctx_unit(g, qc, kt, st):
                nkt = 4 * qc + 4
                if kt == 0:
                    st["c0"] = pctx.tile([65, QC], F32, tag="c", name="ps_c0")
                    st["c1"] = pctx.tile([65, QC], F32, tag="c", name="ps_c1")
                e = st.pop(kt)
                nc.tensor.matmul(
                    st["c0"], vplus[:, kt, 130 * g:130 * g + 65], e[:, 0:QC],
                    start=(kt == 0), stop=(kt == nkt - 1),
                )
                nc.tensor.matmul(
                    st["c1"], vplus[:, kt, 130 * g + 65:130 * g + 130],
                    e[:, QC:2 * QC],
                    start=(kt == 0), stop=(kt == nkt - 1),
                )

            def normalize_unit(g, qc, h, st):
                ps_c = st["c0"] if h == 0 else st["c1"]
                t_d = small.tile([1, QC], F32, tag="den", name="t_d")
                nc.vector.tensor_copy(t_d, ps_c[64:65, :])
                t_r = small.tile([1, QC], F32, tag="rec", name="t_r")
                nc.vector.reciprocal_approx_fast(t_r, t_d)
                t_bc = small.tile([64, QC], F32, tag="bc", name="t_bc")
                nc.gpsimd.partition_broadcast(t_bc, t_r)
                nc.vector.tensor_mul(
                    ctxT[64 * h:64 * (h + 1), g, qc * QC:(qc + 1) * QC],
                    ps_c[0:64, :], t_bc,
                )

            def outproj_unit(qt):
                stg = ostage.tile([128, D], F32, tag="ost", name="stg")
                for ch in range(2):
                    ps_o = pp.tile([128, QC], F32, tag="p", name="ps_o")
                    for g in range(NP):
                        nc.tensor.matmul(
                            ps_o, ctxT[:, g, qt * 128:(qt + 1) * 128],
                            t_wo[:, g, ch * QC:(ch + 1) * QC],
                            start=(g == 0), stop=(g == NP - 1),
                        )
                    if ch == 0:
                        nc.vector.tensor_copy(stg[:, 0:QC], ps_o)
                    else:
                        nc.scalar.copy(stg[:, QC:2 * QC], ps_o)
                nc.sync.dma_start(out[qt * 128:(qt + 1) * 128, :], stg)

            def attn_units(qc):
                """Software-pipelined attention for all 4 pairs.

                Returns (kind, fn) pairs; kind 'c' marks ctx matmuls that
                wait on the scalar exp — the weave slots independent proj /
                out-proj work right before those to hide the latency.
                """
                units = []
                nkt = 4 * qc + 4
                for g in range(NP):
                    st = {}
                    for kt in range(nkt):
                        units.append(('s', lambda g=g, qc=qc, kt=kt, st=st:
                                      attn_scores_unit(g, qc, kt, st)))
                        if kt >= 1:
                            units.append(('c', lambda g=g, qc=qc, kt=kt - 1, st=st:
                                          attn_ctx_unit(g, qc, kt, st)))
                    units.append(('c', lambda g=g, qc=qc, st=st:
                                  attn_ctx_unit(g, qc, nkt - 1, st)))
                    units.append(('n', lambda g=g, qc=qc, st=st: (
                        normalize_unit(g, qc, 0, st),
                        normalize_unit(g, qc, 1, st))))
                return units

            def outproj_units(qc):
                return [lambda qt=qt: outproj_unit(qt)
                        for qt in range(4 * qc, 4 * qc + 4)]

            def weave(a_units, p_units):
                """Emit a_units, slotting each p_unit directly before a ctx
                unit (which stalls on the scalar exp) — evenly spread."""
                ctx_pos = [i for i, (k, _) in enumerate(a_units) if k == 'c']
                assign = {}
                npu = len(p_units)
                if npu and ctx_pos:
                    nslot = len(ctx_pos)
                    nuse = min(npu, nslot)
                    for j in range(nuse):
                        assign.setdefault(ctx_pos[(j * nslot) // nuse], []) \
                              .append(p_units[j])
                    extra = p_units[nuse:]
                else:
                    extra = list(p_units)
                for i, (k, u) in enumerate(a_units):
                    for p in assign.get(i, ()):
                        p()
                    u()
                for p in extra:
                    p()

            # ---------- schedule ----------
            # out-proj for q-chunk qc is deferred into stage qc+1's weave so
            # every attention stage (incl. the scalar-paced last one) has
            # independent tensor work to fill pipeline holes.
            for u in proj_units(0, pool_alt=True):
                u()
            for qc in range(NQC):
                a = attn_units(qc)
                p = []
                if qc >= 1:
                    p += outproj_units(qc - 1)
                if qc + 1 < NQC:
                    p += proj_units(qc + 1)
                weave(a, p)
            for u in outproj_units(NQC - 1):
                u()

    nc.compile()
    return nc


def _host_inputs(x, wq, bq, wk, bk, wv, bv, wo, bo):
    import ml_dtypes
    bf16 = ml_dtypes.bfloat16
    x = np.asarray(x, np.float32)
    wq, wk, wv, wo = (np.asarray(a, np.float32) for a in (wq, wk, wv, wo))
    bq, bk, bv_, bo = (np.asarray(a, np.float32) for a in (bq, bk, bv, bo))

    # causal masks for the 4 diagonal offsets, duplicated for the 2 heads
    p = np.arange(128)[:, None]
    j = np.arange(QC)[None, :]
    cmask = np.zeros((128, 4, 2 * QC), dtype=np.float32)
    for o in range(4):
        m = (j >= p + o * 128).astype(np.float32)
        cmask[:, o, 0:QC] = m
        cmask[:, o, QC:2 * QC] = m
    cmask = cmask.astype(bf16)

    in_maps = []
    for c in range(NCORES):
        b, g = c // 2, c % 2
        hs = slice(g * HG, (g + 1) * HG)
        xt = np.ascontiguousarray(
            x[b].T.reshape(NKD, 128, S).transpose(1, 0, 2)).astype(bf16)
        wqkv = np.concatenate([wq[:, hs], wk[:, hs], wv[:, hs]], axis=1)
        wqkv = np.ascontiguousarray(
            wqkv.reshape(NKD, 128, 3 * HG).transpose(1, 0, 2)).astype(bf16)
        bqk = np.stack([bq[hs].reshape(NP, 128), bk[hs].reshape(NP, 128)],
                       axis=-1)  # [NP, 128, 2]
        bqk = np.ascontiguousarray(bqk.transpose(1, 0, 2))
        bvc = np.ascontiguousarray(bv_[hs][None, :])
        woc = np.ascontiguousarray(
            wo[hs, :].reshape(NP, 128, D).transpose(1, 0, 2)).astype(bf16)
        in_maps.append({
            "xt": xt, "wqkv": wqkv, "bqk": bqk, "bv": bvc,
            "wo": woc, "cmask": cmask,
        })
    return in_maps


def kernel(x, wq, bq, wk, bk, wv, bv, wo, bo, _trace=False, _tmpdir=None):
    if "nc" not in _CACHE:
        _CACHE["nc"] = _build()
    nc = _CACHE["nc"]
    in_maps = _host_inputs(x, wq, bq, wk, bk, wv, bv, wo, bo)
    res = bass_utils.run_bass_kernel_spmd(
        nc, in_maps, core_ids=list(range(NCORES)), trace=_trace, tmpdir=_tmpdir
    )
    _CACHE["last_results"] = res
    bo64 = np.asarray(bo, dtype=np.float64)[None, :]
    outs = []
    for b in range(B):
        acc = (res.results[2 * b]["out"].astype(np.float64)
               + res.results[2 * b + 1]["out"].astype(np.float64) + bo64)
        outs.append(acc.astype(np.float32))
    return np.stack(outs, axis=0)



# revision 8
# speedup vs baseline: 1.1162x; 1.1162x over previous
"""Causal multi-head attention on 8 Trainium2 NeuronCores.

v2: (batch x head-group) sharding — core c handles batch b=c//2 and heads
[8g:8g+8] (g=c%2), i.e. 4 row-packed head PAIRS per core. All matmul
operands are bf16 (fp32r streams at half the PE rate; bf16 runs 1 col/cycle
at 2.4GHz once the p-state ramps). PSUM accumulates in f32.

Per-core layout (partition dim first):
  t_x    [128, 8, 2048]  x_b^T tiled: [p, kd, seq], d = kd*128+p
  t_w    [128, 8, 1536]  wqkv tiles: [p, kd, q512|k512|v512]
  qT/kT  [128, 4, 2048]  pair m: partitions = m's 128 head dims
  vplus  [128, 16, 520]  per key tile: 8 x [v_head(64) | ones(1)]
  scores [128, 1024]     psum, both heads of a pair (row-packed matmuls)
  ctx    [65, 512]       psum per head: rows 0:64 ctx^T, row 64 denom
  ctxT   [128, 4, 2048]  normalized context, bf16
  out    [2048, 1024]    f32 partial (host sums 2 cores + bo per batch)

V is computed directly in transposed orientation (x-tile stationary,
wv moving) so no on-chip transposes are needed; the v bias is folded in
as a K=1 ones-row matmul that initializes the psum accumulator. q/k biases
fold into the psum->SBUF copies (per-partition tensor_scalar_add).

Emission order software-pipelines the attention inner loop (scores kt+1
issued before ctx kt) and weaves projection units for q-chunk qc+1 into
the attention stream of q-chunk qc to keep the tensor engine dense.
"""

import numpy as np
from contextlib import ExitStack

import concourse.bass as bass
import concourse.mybir as mybir
import concourse.tile as tile
from concourse import bacc
from concourse import bass_utils

F32 = mybir.dt.float32
BF16 = mybir.dt.bfloat16
AF = mybir.ActivationFunctionType

B, S, D = 4, 2048, 1024
H, DH = 16, 64
NCORES = 8
HG = 512            # head dims per core (8 heads)
NP = 4              # head pairs per core
QC = 512            # q-chunk width
NQC = S // QC       # 4
NKD = D // 128      # 8 contraction tiles
NKT = S // 128      # 16 key tiles

_CACHE = {}


def _build():
    nc = bacc.Bacc("TRN2", target_bir_lowering=False, debug=False)
    xt = nc.dram_tensor("xt", [128, NKD, S], BF16, kind="ExternalInput").ap()
    wqkv = nc.dram_tensor("wqkv", [128, NKD, 3 * HG], BF16, kind="ExternalInput").ap()
    bqk = nc.dram_tensor("bqk", [128, NP, 2], F32, kind="ExternalInput").ap()
    bv = nc.dram_tensor("bv", [1, HG], F32, kind="ExternalInput").ap()
    wo = nc.dram_tensor("wo", [128, NP, D], BF16, kind="ExternalInput").ap()
    cmask = nc.dram_tensor("cmask", [128, 128], BF16, kind="ExternalInput").ap()
    out = nc.dram_tensor("out", [S, D], F32, kind="ExternalOutput").ap()

    with tile.TileContext(nc) as tc:
        with ExitStack() as ctx:
            consts = ctx.enter_context(tc.tile_pool(name="consts", bufs=1))
            expp = ctx.enter_context(tc.tile_pool(name="expp", bufs=6))
            small = ctx.enter_context(tc.tile_pool(name="small", bufs=2))
            ostage = ctx.enter_context(tc.tile_pool(name="ostage", bufs=2))
            psc = ctx.enter_context(tc.tile_pool(name="psc", bufs=2, space="PSUM"))
            pctx = ctx.enter_context(tc.tile_pool(name="pctx", bufs=2, space="PSUM"))
            pp = ctx.enter_context(tc.tile_pool(name="pp", bufs=2, space="PSUM"))

            # ---- persistent SBUF tensors ----
            t_x = consts.tile([128, NKD, S], BF16, tag="x")
            t_w = consts.tile([128, NKD, 3 * HG], BF16, tag="w")
            t_bqk = consts.tile([128, NP, 2], F32, tag="bqk")
            t_bv = consts.tile([1, HG], F32, tag="bv")
            t_onesc = consts.tile([128, 1], F32, tag="onesc")
            t_bvb = consts.tile([128, HG], F32, tag="bvb")
            t_wo = consts.tile([128, NP, D], BF16, tag="wo")
            t_mask = consts.tile([128, 128], BF16, tag="mask")
            qT = consts.tile([128, NP, S], BF16, tag="qT")
            kT = consts.tile([128, NP, S], BF16, tag="kT")
            vplus = consts.tile([128, NKT, 8 * 65], BF16, tag="vplus")
            ctxT = consts.tile([128, NP, S], BF16, tag="ctxT")

            # input DMAs: interleave weights + x chunk 0 per contraction
            # tile, matching proj(0)'s kd-order consumption so the first
            # matmul starts after ~1 tile instead of the full 7MB.
            nc.sync.dma_start(t_bqk, bqk)
            nc.sync.dma_start(t_bv, bv)
            nc.sync.dma_start(t_mask, cmask)
            for kd in range(NKD):
                nc.sync.dma_start(t_w[:, kd, :], wqkv[:, kd, :])
                nc.sync.dma_start(t_x[:, kd, 0:QC], xt[:, kd, 0:QC])
            nc.sync.dma_start(t_wo, wo)
            for qc in range(1, NQC):
                nc.sync.dma_start(
                    t_x[:, :, qc * QC:(qc + 1) * QC], xt[:, :, qc * QC:(qc + 1) * QC]
                )
            nc.vector.memset(t_onesc, 1.0)
            # broadcast the v bias to all partitions once (keys dim)
            nc.gpsimd.partition_broadcast(t_bvb, t_bv)
            # ones column after each head's 64 v-dims in every key tile
            for hb in range(8):
                nc.vector.tensor_copy(
                    vplus[:, :, 65 * hb + 64:65 * hb + 65],
                    t_onesc[:, None, :].broadcast_to([128, NKT, 1]),
                )

            # ---------- unit emitters ----------
            def proj_qk_unit(m, p, qc, pool, tag):
                ps = pool.tile([128, QC], F32, tag=tag, name="ps_p")
                c0 = p * HG + m * 128
                for kd in range(NKD):
                    nc.tensor.matmul(
                        ps, t_w[:, kd, c0:c0 + 128],
                        t_x[:, kd, qc * QC:(qc + 1) * QC],
                        start=(kd == 0), stop=(kd == NKD - 1),
                    )
                dst = (qT if p == 0 else kT)[:, m, qc * QC:(qc + 1) * QC]
                nc.vector.tensor_scalar_add(dst, ps, t_bqk[:, m, p:p + 1])

            def proj_v_unit(t, qc, pool, tag):
                kt = 4 * qc + t
                ps = pool.tile([128, QC], F32, tag=tag, name="ps_v")
                for kd in range(NKD):
                    nc.tensor.matmul(
                        ps, t_x[:, kd, kt * 128:(kt + 1) * 128],
                        t_w[:, kd, 2 * HG:3 * HG],
                        start=(kd == 0), stop=(kd == NKD - 1),
                    )
                dst = vplus[:, kt, :].rearrange("p (h c) -> p h c", h=8)[:, :, 0:64]
                nc.vector.tensor_add(
                    dst, ps.rearrange("p (h c) -> p h c", h=8),
                    t_bvb.rearrange("p (h c) -> p h c", h=8),
                )

            def proj_units(qc, pool_alt=False):
                units = []
                for m in range(NP):
                    for p in range(2):
                        units.append((proj_qk_unit, m, p, qc))
                for t in range(4):
                    units.append((proj_v_unit, t, qc))
                out_units = []
                for i, u in enumerate(units):
                    fn, *args = u
                    if pool_alt and i % 2 == 1:
                        pool, tag = psc, "s"
                    else:
                        pool, tag = pp, "p"
                    out_units.append(lambda fn=fn, args=args, pool=pool, tag=tag:
                                     fn(*args, pool, tag))
                return out_units

            def attn_scores_unit(g, qc, kt, st):
                # diagonal tiles (o >= 1): queries j < 128*o in this chunk
                # are fully masked for this key tile — trim scores matmul,
                # exp, and the downstream ctx matmul to columns [128*o, QC).
                o = kt - 4 * qc
                tr = 128 * o if o > 0 else 0
                ps_s = psc.tile([128, 2 * QC], F32, tag="s", name="ps_s")
                nc.tensor.matmul(
                    ps_s[:, tr:QC],
                    kT[0:64, g, kt * 128:(kt + 1) * 128],
                    qT[0:64, g, qc * QC + tr:(qc + 1) * QC],
                    start=True, stop=True,
                )
                nc.tensor.matmul(
                    ps_s[:, QC + tr:2 * QC],
                    kT[64:128, g, kt * 128:(kt + 1) * 128],
                    qT[64:128, g, qc * QC + tr:(qc + 1) * QC],
                    start=True, stop=True, tile_position=(64, 0),
                )
                e = expp.tile([128, 2 * QC], BF16, tag="exp", name="t_e")
                if tr:
                    nc.scalar.activation(
                        e.rearrange("p (h w) -> p h w", h=2)[:, :, tr:QC],
                        ps_s.rearrange("p (h w) -> p h w", h=2)[:, :, tr:QC],
                        AF.Exp, scale=0.125,
                    )
                else:
                    nc.scalar.activation(e, ps_s, AF.Exp, scale=0.125)
                if o >= 0:
                    band = e.rearrange("p (h w) -> p h w", h=2)[:, :, tr:tr + 128]
                    nc.vector.tensor_mul(
                        band, band,
                        t_mask[:, None, :].broadcast_to([128, 2, 128]),
                    )
                st[kt] = e

            def attn_ctx_unit(g, qc, kt, st):
                nkt = 4 * qc + 4
                o = kt - 4 * qc
                tr = 128 * o if o > 0 else 0
                if kt == 0:
                    st["c0"] = pctx.tile([65, QC], F32, tag="c", name="ps_c0")
                    st["c1"] = pctx.tile([65, QC], F32, tag="c", name="ps_c1")
                e = st.pop(kt)
                nc.tensor.matmul(
                    st["c0"][:, tr:QC], vplus[:, kt, 130 * g:130 * g + 65],
                    e[:, tr:QC],
                    start=(kt == 0), stop=(kt == nkt - 1),
                )
                nc.tensor.matmul(
                    st["c1"][:, tr:QC], vplus[:, kt, 130 * g + 65:130 * g + 130],
                    e[:, QC + tr:2 * QC],
                    start=(kt == 0), stop=(kt == nkt - 1),
                )

            def normalize_unit(g, qc, h, st):
                ps_c = st["c0"] if h == 0 else st["c1"]
                t_d = small.tile([1, QC], F32, tag="den", name="t_d")
                nc.vector.tensor_copy(t_d, ps_c[64:65, :])
                t_r = small.tile([1, QC], F32, tag="rec", name="t_r")
                nc.vector.reciprocal_approx_fast(t_r, t_d)
                t_bc = small.tile([64, QC], F32, tag="bc", name="t_bc")
                nc.gpsimd.partition_broadcast(t_bc, t_r)
                nc.vector.tensor_mul(
                    ctxT[64 * h:64 * (h + 1), g, qc * QC:(qc + 1) * QC],
                    ps_c[0:64, :], t_bc,
                )

            def outproj_unit(qt):
                stg = ostage.tile([128, D], F32, tag="ost", name="stg")
                for ch in range(2):
                    ps_o = pp.tile([128, QC], F32, tag="p", name="ps_o")
                    for g in range(NP):
                        nc.tensor.matmul(
                            ps_o, ctxT[:, g, qt * 128:(qt + 1) * 128],
                            t_wo[:, g, ch * QC:(ch + 1) * QC],
                            start=(g == 0), stop=(g == NP - 1),
                        )
                    if ch == 0:
                        nc.vector.tensor_copy(stg[:, 0:QC], ps_o)
                    else:
                        nc.scalar.copy(stg[:, QC:2 * QC], ps_o)
                nc.sync.dma_start(out[qt * 128:(qt + 1) * 128, :], stg)

            def attn_units(qc):
                """Software-pipelined attention for all 4 pairs.

                Returns (kind, fn) pairs; kind 'c' marks ctx matmuls that
                wait on the scalar exp — the weave slots independent proj /
                out-proj work right before those to hide the latency.
                """
                units = []
                nkt = 4 * qc + 4
                for g in range(NP):
                    st = {}
                    for kt in range(nkt):
                        units.append(('s', lambda g=g, qc=qc, kt=kt, st=st:
                                      attn_scores_unit(g, qc, kt, st)))
                        if kt >= 1:
                            units.append(('c', lambda g=g, qc=qc, kt=kt - 1, st=st:
                                          attn_ctx_unit(g, qc, kt, st)))
                    units.append(('c', lambda g=g, qc=qc, st=st:
                                  attn_ctx_unit(g, qc, nkt - 1, st)))
                    units.append(('n', lambda g=g, qc=qc, st=st: (
                        normalize_unit(g, qc, 0, st),
                        normalize_unit(g, qc, 1, st))))
                return units

            def outproj_units(qc):
                return [lambda qt=qt: outproj_unit(qt)
                        for qt in range(4 * qc, 4 * qc + 4)]

            def weave(a_units, p_units):
                """Emit a_units, slotting each p_unit directly before a ctx
                unit (which stalls on the scalar exp) — evenly spread."""
                ctx_pos = [i for i, (k, _) in enumerate(a_units) if k == 'c']
                assign = {}
                npu = len(p_units)
                if npu and ctx_pos:
                    nslot = len(ctx_pos)
                    nuse = min(npu, nslot)
                    for j in range(nuse):
                        assign.setdefault(ctx_pos[(j * nslot) // nuse], []) \
                              .append(p_units[j])
                    extra = p_units[nuse:]
                else:
                    extra = list(p_units)
                for i, (k, u) in enumerate(a_units):
                    for p in assign.get(i, ()):
                        p()
                    u()
                for p in extra:
                    p()

            # ---------- schedule ----------
            # out-proj for q-chunk qc is deferred into stage qc+1's weave so
            # every attention stage (incl. the scalar-paced last one) has
            # independent tensor work to fill pipeline holes.
            for u in proj_units(0, pool_alt=True):
                u()
            for qc in range(NQC):
                a = attn_units(qc)
                p = []
                if qc >= 1:
                    p += outproj_units(qc - 1)
                if qc + 1 < NQC:
                    p += proj_units(qc + 1)
                weave(a, p)
            for u in outproj_units(NQC - 1):
                u()

    nc.compile()
    return nc


def _host_inputs(x, wq, bq, wk, bk, wv, bv, wo, bo):
    import ml_dtypes
    bf16 = ml_dtypes.bfloat16
    x = np.asarray(x, np.float32)
    wq, wk, wv, wo = (np.asarray(a, np.float32) for a in (wq, wk, wv, wo))
    bq, bk, bv_, bo = (np.asarray(a, np.float32) for a in (bq, bk, bv, bo))

    # single 128x128 causal triangle band: every diagonal key tile sees
    # the same local pattern keep[jj >= p] once trimmed to its band.
    p = np.arange(128)[:, None]
    jj = np.arange(128)[None, :]
    cmask = (jj >= p).astype(np.float32).astype(bf16)

    in_maps = []
    for c in range(NCORES):
        b, g = c // 2, c % 2
        hs = slice(g * HG, (g + 1) * HG)
        xt = np.ascontiguousarray(
            x[b].T.reshape(NKD, 128, S).transpose(1, 0, 2)).astype(bf16)
        wqkv = np.concatenate([wq[:, hs], wk[:, hs], wv[:, hs]], axis=1)
        wqkv = np.ascontiguousarray(
            wqkv.reshape(NKD, 128, 3 * HG).transpose(1, 0, 2)).astype(bf16)
        bqk = np.stack([bq[hs].reshape(NP, 128), bk[hs].reshape(NP, 128)],
                       axis=-1)  # [NP, 128, 2]
        bqk = np.ascontiguousarray(bqk.transpose(1, 0, 2))
        bvc = np.ascontiguousarray(bv_[hs][None, :])
        woc = np.ascontiguousarray(
            wo[hs, :].reshape(NP, 128, D).transpose(1, 0, 2)).astype(bf16)
        in_maps.append({
            "xt": xt, "wqkv": wqkv, "bqk": bqk, "bv": bvc,
            "wo": woc, "cmask": cmask,
        })
    return in_maps


def kernel(x, wq, bq, wk, bk, wv, bv, wo, bo, _trace=False, _tmpdir=None):
    if "nc" not in _CACHE:
        _CACHE["nc"] = _build()
    nc = _CACHE["nc"]
    in_maps = _host_inputs(x, wq, bq, wk, bk, wv, bv, wo, bo)
    res = bass_utils.run_bass_kernel_spmd(
        nc, in_maps, core_ids=list(range(NCORES)), trace=_trace, tmpdir=_tmpdir
    )
    _CACHE["last_results"] = res
    bo64 = np.asarray(bo, dtype=np.float64)[None, :]
    outs = []
    for b in range(B):
        acc = (res.results[2 * b]["out"].astype(np.float64)
               + res.results[2 * b + 1]["out"].astype(np.float64) + bo64)
        outs.append(acc.astype(np.float32))
    return np.stack(outs, axis=0)

